# revision 29
# baseline (speedup 1.0000x reference)
"""Trainium2 Bass kernel: causal MHA block (B=2, S=2048, D=4096, 32 heads x 128,
fp32 I/O, interleaved RoPE), tensor-parallel over heads on 8 NeuronCores with
AllToAll collectives into a sequence-parallel output projection.

Key design points (vs. the fp32r baseline, 2.00ms -> 1.46ms):
- all PE-facing data is bf16: same PE rate as fp32r but half the HBM bytes and
  half the DVE element time; rel err ~4e-3 vs the 2e-2 gate.
- single pass over x computes Q, K (feature-major, RoPE fused via a pair-swap
  permutation matmul + DVE combine) and V (token-major) with all three weights
  SBUF-resident; weights and x split into lo/hi half-tiles so the first chains
  start after ~2MB of DMA instead of ~8MB.
- attention per (batch, head): K-stationary transposed scores, exp on ScalarE
  (PSUM fp32 -> SBUF bf16), multiplicative 0/1 causal mask on DVE, softmax
  denominator via a ones-column matmul chain, PV chain, normalization through
  a fast-approx reciprocal + broadcast matmul.  dsum/pv chains are emitted
  through a generator and interleaved 2-per-score-matmul so the PE never
  starves behind ScalarE's exp stream; spare yields keep the broadcast matmul
  clear of the reciprocal's DVE latency.
- one PSUM pool set for the whole kernel (QKV chains, scores, and WO
  accumulators share one 4-buf tag ring) -- pool-scope transitions otherwise
  cost ~12-15us each in drain barriers; a small SBUF "bridge" pool outside the
  phase scopes holds the first head's q/k so their loads don't wait on SBUF
  address reuse.
- collectives: batch-0 A2A fires after batch-0 attention (hidden under
  batch-1); batch-1's A2A is split 3 ways (heads 0-1 / 2 / 3) firing as soon
  as each head's output is staged, with separate contiguous DRAM buffers and
  separate at_sb tiles per piece so WO's accumulation (ordered g-major over
  feature tiles) starts before the later pieces land.
- WO reads wo exactly once (both batches share each wo tile), with the first
  two tiles prefetched mid-attention via tile_wait_until; batch-0 chains and
  early batch-1 segments buffer ~50us of PE work against collective jitter.
- every DMA is a large merged transfer (1-32KB per partition row); DMA issue
  is spread across the sync/scalar/gpsimd queues so no queue's semaphore wait
  can stall another phase's stream (the Tile scheduler reorders zero-dep DMAs,
  so the critical wq load is pinned with high_priority).
"""

import contextlib
import sys

if "/opt/trn_rl_repo" not in sys.path:
    sys.path.insert(0, "/opt/trn_rl_repo")

import numpy as np

import concourse.bass as bass
import concourse.tile as tile
from concourse import bacc, mybir
from concourse.bass_utils import run_bass_kernel_spmd

F32 = mybir.dt.float32
BF16 = mybir.dt.bfloat16

B, S, D = 2, 2048, 4096
H, HD = 32, 128
NCORES = 8
HPC = H // NCORES        # heads per core
F = HPC * HD             # 512 features per core
TOK = B * S              # 4096 tokens
KT = D // 128            # 32 contraction tiles
NB = TOK // 512          # 8 token blocks of 512
SCALE = 1.0 / float(np.sqrt(HD))

_CACHE = {}


def _build():
    nc = bacc.Bacc("TRN2", target_bir_lowering=False, debug=False,
                   num_devices=NCORES)

    x_d = nc.dram_tensor("xt", [NB, 2, 128, 16 * 512], BF16,
                         kind="ExternalInput")
    wq_d = nc.dram_tensor("wqT", [128, KT * F], BF16, kind="ExternalInput")
    wk_d = nc.dram_tensor("wkT", [128, KT * F], BF16, kind="ExternalInput")
    wv_d = nc.dram_tensor("wvT", [128, KT * F], BF16, kind="ExternalInput")
    wo_d = nc.dram_tensor("woT", [D // 512, 128, KT * 512], BF16,
                          kind="ExternalInput")
    cos_d = nc.dram_tensor("cosE", [128, S], BF16, kind="ExternalInput")
    sin_d = nc.dram_tensor("sinE", [128, S], BF16, kind="ExternalInput")
    tri_d = nc.dram_tensor("tri01", [128, 4 * 512], BF16, kind="ExternalInput")
    perm_d = nc.dram_tensor("permT", [128, 128], BF16, kind="ExternalInput")
    ones_d = nc.dram_tensor("ones", [128, 128], BF16, kind="ExternalInput")
    out_d = nc.dram_tensor("out", [TOK // NCORES, D], F32,
                           kind="ExternalOutput")

    with tile.TileContext(nc) as tc:
        dram = tc.alloc_tile_pool(name="dram", bufs=1, space="DRAM")
        q_sp = [dram.tile([HPC, 128, S], BF16, name=f"q_sp{b}")
                for b in range(B)]
        k_sp = [dram.tile([HPC, 128, S], BF16, name=f"k_sp{b}")
                for b in range(B)]
        v_sp = [dram.tile([128, (S // 128) * F], BF16, name=f"v_sp{b}")
                for b in range(B)]
        a2a_in = [dram.tile([NCORES, F, 256], BF16, name="a2a_in0"),
                  dram.tile([NCORES, F // 2, 256], BF16, name="a2a_in1a"),
                  dram.tile([NCORES, F // 4, 256], BF16, name="a2a_in1b"),
                  dram.tile([NCORES, F // 4, 256], BF16, name="a2a_in1c")]
        a2a_out = [dram.tile([NCORES, F, 256], BF16, name="a2a_out0"),
                   dram.tile([NCORES, F // 2, 256], BF16, name="a2a_out1a"),
                   dram.tile([NCORES, F // 4, 256], BF16, name="a2a_out1b"),
                   dram.tile([NCORES, F // 4, 256], BF16, name="a2a_out1c")]

        with tc.tile_pool(name="consts", bufs=1) as cpool:
            perm_sb = cpool.tile([128, 128], BF16)
            nc.sync.dma_start(out=perm_sb[:], in_=perm_d[:, :])
            ones_sb = cpool.tile([128, 128], BF16)
            nc.sync.dma_start(out=ones_sb[:], in_=ones_d[:, :])
            cos_sb = cpool.tile([128, S], BF16)
            nc.sync.dma_start(out=cos_sb[:], in_=cos_d[:, :])
            sin_sb = cpool.tile([128, S], BF16)
            nc.sync.dma_start(out=sin_sb[:], in_=sin_d[:, :])
            tri_sb = cpool.tile([128, 4 * 512], BF16)
            nc.sync.dma_start(out=tri_sb[:], in_=tri_d[:, :])

            # one PSUM pool set for the whole kernel (no pool-scope
            # transitions, which cost ~12us of drain each): tag "sc" ring
            # of 4 serves QKV chains, attention scores and WO accumulators;
            # tag "pv" ring of 2 serves RoPE rotations and PV chains
            _ps = contextlib.ExitStack()
            psA = _ps.enter_context(
                tc.tile_pool(name="psA", bufs=4, space="PSUM"))
            psB = _ps.enter_context(
                tc.tile_pool(name="psB", bufs=2, space="PSUM"))
            dps = _ps.enter_context(
                tc.tile_pool(name="dps", bufs=1, space="PSUM"))
            bcps = _ps.enter_context(
                tc.tile_pool(name="bcps", bufs=1, space="PSUM"))
            # small SBUF pool outside the phase scopes: batch-0 head-0 q/k
            # land here so their loads don't wait for the QKV pools'
            # addresses to free (SBUF address reuse otherwise serializes the
            # phase transition on the last QKV matmul)
            bridge = _ps.enter_context(tc.tile_pool(name="bridge", bufs=1))

            # ======== single pass over x: Q, K (feature-major + RoPE) and V
            # (token-major), all three weights SBUF-resident in bf16
            with tc.tile_pool(name="wpool", bufs=1) as wpool, \
                 tc.tile_pool(name="xpool", bufs=2) as xpool, \
                 tc.tile_pool(name="qkvw", bufs=2) as work:

                w_sb = {}
                half_c = KT * F // 2
                for nm, w_d in (("q", wq_d), ("k", wk_d), ("v", wv_d)):
                    # lo/hi separate tiles: the first 16 contraction tiles of
                    # the first chains only wait on the lo half (half the
                    # bytes) instead of the whole weight
                    lo = wpool.tile([128, half_c], BF16, tag=f"w{nm}l",
                                    name=f"w{nm}l")
                    hi = wpool.tile([128, half_c], BF16, tag=f"w{nm}h",
                                    name=f"w{nm}h")
                    if nm == "q":
                        # the first QK chain waits on wq: force it to the
                        # front of both DMA queues (the scheduler orders
                        # zero-dep DMAs arbitrarily otherwise)
                        with tc.high_priority():
                            nc.sync.dma_start(out=lo[:], in_=w_d[:, :half_c])
                            nc.scalar.dma_start(out=hi[:], in_=w_d[:, half_c:])
                    else:
                        nc.sync.dma_start(out=lo[:], in_=w_d[:, :half_c])
                        nc.scalar.dma_start(out=hi[:], in_=w_d[:, half_c:])
                    w_sb[nm] = (lo, hi)

                for nb in range(NB):
                    xhs = []
                    for half in range(2):
                        xt = xpool.tile([128, half_c], BF16, tag=f"x{half}",
                                        name=f"x{half}")
                        nc.gpsimd.dma_start(out=xt[:], in_=x_d[nb, half, :, :])
                        xhs.append(xt)
                    pos = (nb % (S // 512)) * 512
                    for m in range(2 * HPC):
                        wt = w_sb["q"] if m < HPC else w_sb["k"]
                        o_sp = q_sp if m < HPC else k_sp
                        h = m % HPC
                        ps = psA.tile([128, 512], F32, tag="sc",
                                      name="ps")
                        for kt in range(KT):
                            hf, kk = divmod(kt, 16)
                            nc.tensor.matmul(
                                ps[:],
                                wt[hf][:, kk * F + h * 128:
                                       kk * F + (h + 1) * 128],
                                xhs[hf][:, kk * 512:(kk + 1) * 512],
                                start=(kt == 0), stop=(kt == KT - 1))
                        raw = work.tile([128, 512], BF16, tag="raw",
                                        name="raw")
                        nc.scalar.copy(raw[:], ps[:])
                        rot = psB.tile([128, 512], F32, tag="pv",
                                       name="rot")
                        nc.tensor.matmul(rot[:], perm_sb[:], raw[:],
                                         start=True, stop=True)
                        t1 = work.tile([128, 512], F32, tag="t1", name="t1")
                        nc.vector.tensor_mul(t1[:], raw[:],
                                             cos_sb[:, pos:pos + 512])
                        t2 = work.tile([128, 512], F32, tag="t2", name="t2")
                        nc.vector.tensor_mul(t2[:], rot[:],
                                             sin_sb[:, pos:pos + 512])
                        qf = work.tile([128, 512], BF16, tag="qf", name="qf")
                        nc.vector.tensor_add(qf[:], t1[:], t2[:])
                        nc.sync.dma_start(
                            out=o_sp[nb // 4][h, :,
                                              (nb % 4) * 512:
                                              (nb % 4 + 1) * 512],
                            in_=qf[:])
                    for ts in range(4):
                        ps = psA.tile([128, 512], F32, tag="sc",
                                      name="psv")
                        for kt in range(KT):
                            hf, kk = divmod(kt, 16)
                            nc.tensor.matmul(
                                ps[:],
                                xhs[hf][:, kk * 512 + ts * 128:
                                        kk * 512 + (ts + 1) * 128],
                                w_sb["v"][hf][:, kk * F:(kk + 1) * F],
                                start=(kt == 0), stop=(kt == KT - 1))
                        vf = work.tile([128, 512], BF16, tag="vf", name="vf")
                        nc.vector.tensor_copy(vf[:], ps[:])
                        st_g = nb * 4 + ts
                        nc.sync.dma_start(
                            out=v_sp[st_g // 16][:,
                                                 (st_g % 16) * F:
                                                 (st_g % 16 + 1) * F],
                            in_=vf[:])

            # ======== attention + per-batch AllToAll, then single-pass WO
            with tc.tile_pool(name="aqk", bufs=2) as apool, \
                 tc.tile_pool(name="avp", bufs=2) as vpool, \
                 tc.tile_pool(name="exw", bufs=26) as expool, \
                 tc.tile_pool(name="amisc", bufs=2) as misc, \
                 tc.tile_pool(name="atsb", bufs=1) as atsb, \
                 tc.tile_pool(name="wop", bufs=2) as wopool, \
                 tc.tile_pool(name="wout", bufs=3) as wout:

                # wo n=0,1 prefetched on sync, pinned (via the scheduler's
                # model clock) to mid-attention: early enough to hide the
                # transfer, late enough not to crowd the weight/x streams or
                # the collectives
                wo_tiles = {}
                for n in range(2):
                    t = wopool.tile([128, KT * 512], BF16, tag="wo",
                                    name="wo_sb")
                    with tc.tile_wait_until(0.60 + 0.04 * n):
                        nc.sync.dma_start(out=t[:], in_=wo_d[n, :, :])
                    wo_tiles[n] = t

                at_sb = [None, None, None, None]

                if True:

                    def finish_gen(b, h, qt, exs, v_sb):
                        # dsum/pv chains as a generator: yields after each
                        # matmul so the caller can interleave them between
                        # the next qt's score matmuls (keeps PE fed while
                        # ScalarE works through the exp backlog)
                        nkt = 4 * qt + 4
                        dsum = dps.tile([1, 512], F32, name="dsum",
                                        tag="dsum")
                        for kt in range(nkt):
                            nc.tensor.matmul(
                                dsum[:], ones_sb[:, 0:1], exs[kt][:],
                                start=(kt == 0), stop=(kt == nkt - 1))
                            yield
                        rec32 = misc.tile([1, 512], F32, tag="rec32",
                                          name="rec32")
                        nc.vector.reciprocal_approx_fast(out=rec32[:],
                                                         in_=dsum[:])
                        rec = misc.tile([1, 512], BF16, tag="rec", name="rec")
                        with nc.allow_low_precision(
                                reason="1/denom feeds bf16 matmul"):
                            nc.vector.tensor_copy(rec[:], rec32[:])
                        pv = psB.tile([128, 512], F32, tag="pv",
                                      name="pv")
                        for kt in range(nkt):
                            nc.tensor.matmul(
                                pv[:],
                                v_sb[:, kt * F + h * 128:
                                     kt * F + (h + 1) * 128],
                                exs[kt][:],
                                start=(kt == 0), stop=(kt == nkt - 1))
                            yield
                        # a few spare yields so the bc matmul (which waits on
                        # the DVE reciprocal) lands well after it completes
                        for _ in range(5):
                            yield
                        bc = bcps.tile([128, 512], F32, name="bc")
                        nc.tensor.matmul(bc[:], ones_sb[0:1, :], rec[:],
                                         start=True, stop=True)
                        bc_sb = misc.tile([128, 512], BF16, tag="bcsb",
                                          name="bc_sb")
                        nc.vector.tensor_copy(bc_sb[:], bc[:])
                        at = misc.tile([128, 512], BF16, tag="at", name="at")
                        nc.vector.tensor_mul(at[:], pv[:], bc_sb[:])
                        tgt = (a2a_in[0] if b == 0
                               else a2a_in[1] if h < 2 else a2a_in[h])
                        row = (h if b == 0 else h if h < 2 else 0) * 128
                        for u in range(2):
                            nc.sync.dma_start(
                                out=tgt[2 * qt + u, row:row + 128, :],
                                in_=at[:, u * 256:(u + 1) * 256])

                    for b in range(B):
                        # batch-0 attention inputs load on the (idle) gpsimd
                        # queue as soon as the QKV spills land; batch-1's go
                        # on scalar because gpsimd is blocked by the first
                        # collective by then
                        ld = nc.gpsimd if b == 0 else nc.scalar
                        pending = None
                        v_sb = None
                        for h in range(HPC):
                            pool = bridge if (b == 0 and h == 0) else apool
                            q_sb = pool.tile([128, S], BF16, tag="q",
                                             name="q_sb")
                            ld.dma_start(
                                out=q_sb[:], in_=q_sp[b][h, :, :])
                            k_sb = pool.tile([128, S], BF16, tag="k",
                                             name="k_sb")
                            ld.dma_start(
                                out=k_sb[:], in_=k_sp[b][h, :, :])
                            if v_sb is None:
                                v_sb = vpool.tile([128, (S // 128) * F], BF16,
                                                  tag="v", name="v_sb")
                                ld.dma_start(out=v_sb[:], in_=v_sp[b][:, :])

                            for qt in range(4):
                                nkt = 4 * qt + 4
                                exs = []
                                for kt in range(nkt):
                                    sc = psA.tile([128, 512], F32,
                                                  tag="sc", name="sc")
                                    nc.tensor.matmul(
                                        sc[:],
                                        k_sb[:, kt * 128:(kt + 1) * 128],
                                        q_sb[:, qt * 512:(qt + 1) * 512],
                                        start=True, stop=True)
                                    ex = expool.tile([128, 512], BF16,
                                                     tag="ex", name="ex")
                                    nc.scalar.activation(
                                        ex[:], sc[:],
                                        mybir.ActivationFunctionType.Exp,
                                        scale=SCALE)
                                    r = kt - 4 * qt
                                    if r >= 0:
                                        exm = expool.tile(
                                            [128, 512], BF16, tag="ex",
                                            name="exm")
                                        nc.vector.tensor_mul(
                                            exm[:], ex[:],
                                            tri_sb[:, r * 512:(r + 1) * 512])
                                        ex = exm
                                    exs.append(ex)
                                    if pending is not None:
                                        for _ in range(2):
                                            if next(pending, "end") == "end":
                                                pending = None
                                                break
                                if pending is not None:
                                    for _ in pending:
                                        pass
                                pending = finish_gen(b, h, qt, exs, v_sb)
                            if b == 1 and h == 2:
                                # head-2 A2A (quarter) right after its chains
                                for _ in pending:
                                    pass
                                pending = None
                                nc.gpsimd.collective_compute(
                                    "AllToAll", mybir.AluOpType.bypass,
                                    replica_groups=[list(range(NCORES))],
                                    ins=[a2a_in[2][:]], outs=[a2a_out[2][:]])
                                t = atsb.tile([128, KT * 64], BF16,
                                              tag="at1b", name="at_sb1b")
                                nc.gpsimd.dma_start(
                                    out=t[:, :]
                                    .rearrange("p (j t) -> p j t", j=8),
                                    in_=a2a_out[2][:, :, :]
                                    .rearrange("j p t -> p j t"))
                                at_sb[2] = t
                            if b == 1 and h == 1:
                                # half-A2A for batch 1 (heads 0-1 of every
                                # core) fired mid-batch so WO's accumulation
                                # can start on these features while the
                                # second half is still in flight
                                for _ in pending:
                                    pass
                                pending = None
                                nc.gpsimd.collective_compute(
                                    "AllToAll", mybir.AluOpType.bypass,
                                    replica_groups=[list(range(NCORES))],
                                    ins=[a2a_in[1][:]], outs=[a2a_out[1][:]])
                                t = atsb.tile([128, KT * 128], BF16,
                                              tag="at1a", name="at_sb1a")
                                for g in range(2):
                                    nc.gpsimd.dma_start(
                                        out=t[:, g * 2048:(g + 1) * 2048]
                                        .rearrange("p (j t) -> p j t", j=8),
                                        in_=a2a_out[1][:, g * 128:
                                                       (g + 1) * 128, :]
                                        .rearrange("j p t -> p j t"))
                                at_sb[1] = t
                        if pending is not None:
                            for _ in pending:
                                pass
                        if b == 0:
                            nc.gpsimd.collective_compute(
                                "AllToAll", mybir.AluOpType.bypass,
                                replica_groups=[list(range(NCORES))],
                                ins=[a2a_in[0][:]], outs=[a2a_out[0][:]])
                            t = atsb.tile([128, KT * 256], BF16, tag="at0",
                                          name="at_sb0")
                            for g in range(4):
                                nc.gpsimd.dma_start(
                                    out=t[:, g * 2048:(g + 1) * 2048]
                                    .rearrange("p (j t) -> p j t", j=8),
                                    in_=a2a_out[0][:, g * 128:(g + 1) * 128, :]
                                    .rearrange("j p t -> p j t"))
                            at_sb[0] = t
                        else:
                            nc.gpsimd.collective_compute(
                                "AllToAll", mybir.AluOpType.bypass,
                                replica_groups=[list(range(NCORES))],
                                ins=[a2a_in[3][:]], outs=[a2a_out[3][:]])
                            t = atsb.tile([128, KT * 64], BF16,
                                          tag="at1c", name="at_sb1c")
                            nc.gpsimd.dma_start(
                                out=t[:, :]
                                .rearrange("p (j t) -> p j t", j=8),
                                in_=a2a_out[3][:, :, :]
                                .rearrange("j p t -> p j t"))
                            at_sb[3] = t

                    # ---- WO inside the same PSUM scope (pss reuses the
                    # "sc" tag ring) so no pool-transition barrier separates
                    # attention from the output projection.
                    # feature tiles in halves order: g 0-1 (delivered by the
                    # first half-A2A of batch 1) before g 2-3, so batch-1
                    # chains can begin before the second half lands
                    tile_order = ([(j, g) for g in (0, 1) for j in range(8)]
                                  + [(j, g) for g in (2, 3) for j in range(8)])

                    def at_slice(b, j, g, mt):
                        # batch 0: one tile, columns g-major; batch 1: two
                        # half tiles (g 0-1 from the first half-A2A, g 2-3
                        # from the second) so early WO chains never wait on
                        # the later collective
                        if b == 0:
                            src_t, col = at_sb[0], g * 2048 + j * 256
                        elif g < 2:
                            src_t, col = at_sb[1], g * 2048 + j * 256
                        else:
                            src_t, col = at_sb[g], j * 256
                        return src_t[:, col + mt * 128: col + (mt + 1) * 128]

                    SEGS = {None: (0, 32), 0: (0, 16), 1: (16, 24),
                            2: (24, 32)}

                    def wo_emit(n, b, wo_sb, half):
                        # half: None=all tiles, else a causal segment of
                        # tile_order gated by successively later A2A pieces
                        lo, hi = SEGS[half]
                        pss = wo_pss[(n, b)]
                        for idx in range(lo, hi):
                            j, g = tile_order[idx]
                            kt = j * 4 + g
                            for mt in range(2):
                                nc.tensor.matmul(
                                    pss[mt][:], at_slice(b, j, g, mt),
                                    wo_sb[:, kt * 512:(kt + 1) * 512],
                                    start=(idx == 0), stop=(idx == KT - 1))
                        if half in (0, 1):
                            return
                        for mt in range(2):
                            o_sb = wout.tile([128, 512], F32, name="o_sb")
                            nc.scalar.copy(o_sb[:], pss[mt][:])
                            nc.sync.dma_start(
                                out=out_d[b * 256 + mt * 128:
                                          b * 256 + (mt + 1) * 128,
                                          n * 512:(n + 1) * 512],
                                in_=o_sb[:])
                        del wo_pss[(n, b)]

                    wo_pss = {}

                    def wo_chain(n, b, wo_sb, half=None):
                        if (n, b) not in wo_pss:
                            wo_pss[(n, b)] = [
                                psA.tile([128, 512], F32, tag="sc",
                                         name="psw") for mt in range(2)]
                        wo_emit(n, b, wo_sb, half)

                    def get_wo(n):
                        if n in wo_tiles:
                            return wo_tiles.pop(n)
                        t = wopool.tile([128, KT * 512], BF16, tag="wo",
                                        name="wo_sb")
                        nc.sync.dma_start(out=t[:], in_=wo_d[n, :, :])
                        return t

                    # batch-0 chains for n=0,1 first (only need the
                    # long-finished first A2A), then both b1 chains' g01
                    # halves (first half-A2A), then their g23 halves -- by
                    # which point the second half-A2A has landed
                    live = {n: get_wo(n) for n in range(2)}
                    wo_chain(0, 0, live[0])
                    wo_chain(1, 0, live[1])
                    wo_chain(0, 1, live[0], half=0)
                    wo_chain(1, 1, live[1], half=0)
                    wo_chain(0, 1, live[0], half=1)
                    wo_chain(1, 1, live[1], half=1)
                    wo_chain(0, 1, live[0], half=2)
                    wo_chain(1, 1, live[1], half=2)
                    del live[0], live[1]
                    for n in range(2, D // 512):
                        live[n] = get_wo(n)
                        wo_chain(n, 0, live[n])
                        wo_chain(n, 1, live[n])
                        del live[n]

            _ps.close()

    nc.compile()
    return nc


def _host_inputs(x, wq, wk, wv, wo):
    import ml_dtypes
    BF = ml_dtypes.bfloat16

    x = np.asarray(x, dtype=np.float32).reshape(TOK, D)
    # xt[nb, half, p, kk*512+t] = x[nb*512+t, half*2048+kk*128+p]
    xt = np.ascontiguousarray(
        x.T.reshape(2, 16, 128, NB, 512).transpose(3, 0, 2, 1, 4)
        .reshape(NB, 2, 128, 16 * 512)).astype(BF)

    # woT[n, p, kt*512+o] = wo[n*512+o, kt*128+p]
    wot = np.ascontiguousarray(
        np.asarray(wo, np.float32).T.reshape(KT, 128, D // 512, 512)
        .transpose(2, 1, 0, 3).reshape(D // 512, 128, KT * 512)).astype(BF)

    inv = (1.0 / (10000.0 ** (np.arange(0, HD, 2, dtype=np.float64) / HD)))
    fr = np.outer(np.arange(S, dtype=np.float64), inv)        # [S, HD/2]
    cosE = np.repeat(np.cos(fr).T, 2, axis=0).astype(BF)      # [128, S]
    sinE = np.repeat(np.sin(fr).T, 2, axis=0).astype(BF)

    # tri01[p, r*512+q] = 1 where r*128+p <= q (causal keep), else 0
    tri = np.zeros([128, 4 * 512], dtype=np.float32)
    qi = np.arange(512)
    pi = np.arange(128)
    for r in range(4):
        tri[:, r * 512:(r + 1) * 512][
            (r * 128 + pi)[:, None] <= qi[None, :]] = 1.0
    tri = tri.astype(BF)

    permT = np.zeros([128, 128], dtype=np.float32)
    ii = np.arange(0, 128, 2)
    permT[ii + 1, ii] = -1.0
    permT[ii, ii + 1] = 1.0
    permT = permT.astype(BF)

    ones = np.ones([128, 128], dtype=BF)

    def wtile(w, i):
        # [p, kt*512+f] = w[i*512+f, kt*128+p]
        sl = np.asarray(w, np.float32)[i * F:(i + 1) * F, :]
        return np.ascontiguousarray(
            sl.T.reshape(KT, 128, F).transpose(1, 0, 2)
            .reshape(128, KT * F)).astype(BF)

    maps = []
    for i in range(NCORES):
        maps.append(dict(
            xt=xt,
            wqT=wtile(wq, i), wkT=wtile(wk, i), wvT=wtile(wv, i),
            woT=wot, cosE=cosE, sinE=sinE, tri01=tri, permT=permT,
            ones=ones,
        ))
    return maps


def kernel(x, start_pos, wq, wk, wv, wo, _trace=False):
    if "nc" not in _CACHE:
        _CACHE["nc"] = _build()
    nc = _CACHE["nc"]
    maps = _host_inputs(x, wq, wk, wv, wo)
    res = run_bass_kernel_spmd(nc, maps, core_ids=list(range(NCORES)),
                               trace=_trace)
    _CACHE["last"] = res
    full = np.empty([TOK, D], dtype=np.float32)
    for j in range(NCORES):
        o = res.results[j]["out"]
        full[j * 256:(j + 1) * 256] = o[:256]
        full[S + j * 256: S + (j + 1) * 256] = o[256:]
    return full.reshape(B, S, D)
